# revision 35
# baseline (speedup 1.0000x reference)
"""Multi-head self-attention (B=2, S=2048, E=1024, H=16, causal) on 8 trn2 cores.

Sharding: core c handles batch b = c // 4 and heads [4*(c%4), 4*(c%4)+4).
Each core computes its 4 heads' attention and a partial output projection
(row-sharded Wout); the host sums the 4 partials per batch and adds bout.

Key optimizations over the naive version:
- qt loaded S-major in four 1MB blocks so the first projection matmuls start
  ~6us in instead of waiting for the full 4MB.
- Attention processes head PAIRS: the two heads of a pair live in SBUF
  partitions 0-63 / 64-127, so their K=64 score matmuls land on different
  PE row-groups (tile_position (0,0) vs (64,0)) and stream concurrently.
- exp is split across ScalarE (exact activation) and VectorE (Schraudolph
  fast-exp: bits = round(x*128/ln2 + B) written as int16 = bf16 bits).
- V stationary padded to 128 columns (ones in 64..127) so LDWEIGHTS runs
  with FWL; column 64 doubles as the softmax-denominator ones column.
- Output projection partials stored/DMAd as bf16 (halves output traffic).
"""

import os
from contextlib import ExitStack

import ml_dtypes
import numpy as np

import concourse.bass as bass
import concourse.mybir as mybir
import concourse.tile as tile
from concourse import bacc
from concourse.bass_utils import run_bass_kernel_spmd

f32 = mybir.dt.float32
bf16 = mybir.dt.bfloat16
i16 = mybir.dt.int16
bfnp = ml_dtypes.bfloat16

S = 2048
E = 1024
HC = 4  # heads per core
D = 64
C = HC * D  # 256 per-core head dims
NE = E // 128  # 8 contraction chunks

Exp = mybir.ActivationFunctionType.Exp
Ident = mybir.ActivationFunctionType.Identity
Mult = mybir.AluOpType.mult
Add = mybir.AluOpType.add

# Schraudolph fast-exp constants in bf16-bit domain
A16 = 128.0 / np.log(2.0)
B16C = 127.0 * 128.0 - 7.0


def _build_kernel(tc, qt, wq, wk, wv, wo, bq, bk, bv, y):
    nc = tc.nc
    rrow = nc.dram_tensor("rrow", [16, 512], f32).ap()
    with ExitStack() as ctx:
        const = ctx.enter_context(tc.tile_pool(name="const", bufs=1))
        qt_sb = const.tile([128, 4, NE, 512], bf16)
        wq_sb = const.tile([128, NE, C], bf16)
        wk_sb = const.tile([128, NE, C], bf16)
        wv_sb = const.tile([128, NE, C], bf16)
        wo_sb = const.tile([128, 2, E], bf16)
        bq_sb = const.tile([128, 2], f32)
        bk_sb = const.tile([128, 2], f32)
        bv_sb = const.tile([1, C], bf16)
        ones_sb = const.tile([1, 128], bf16)
        trim_sb = const.tile([128, 128], bf16)
        ident_sb = const.tile([128, 128], bf16)
        qT_sb = const.tile([128, 2, S], bf16)
        kT_sb = const.tile([128, 2, S], bf16)
        v_sb = [
            const.tile([128, HC, 128], bf16, tag=f"v{si}", name=f"v_sb{si}")
            for si in range(16)
        ]
        out_sb = const.tile([128, 2, S], bf16)

        # --- loads: weights first (parallel queues), qt S-major on sync ---
        nc.scalar.dma_start(wq_sb[:], wq.rearrange("(p a) -> p a", p=128))
        nc.gpsimd.dma_start(wk_sb[:], wk.rearrange("(p a) -> p a", p=128))
        qt_r = qt.rearrange("(g p i c) -> g p i c", g=4, p=128, i=NE)
        nc.sync.dma_start(qt_sb[:, 0], qt_r[0])
        nc.sync.dma_start(bq_sb[:], bq[:])
        nc.sync.dma_start(bk_sb[:], bk[:])
        nc.sync.dma_start(bv_sb[:], bv[:])
        for g in range(1, 4):
            nc.sync.dma_start(qt_sb[:, g], qt_r[g])
        nc.scalar.dma_start(wv_sb[:], wv.rearrange("(p a) -> p a", p=128))
        nc.gpsimd.dma_start(wo_sb[:], wo.rearrange("(p a) -> p a", p=128))
        nc.vector.memset(ones_sb[:], 1.0)
        for si in range(16):
            nc.gpsimd.memset(v_sb[si][:, :, D:128], 1.0)
        # constant tiles for matmul-based causal masking of diagonal blocks:
        # trim[j, k] = -1e5 iff j < k; ident = I. Then trim.T @ I accumulated
        # into a 128-col score block adds -1e5 wherever q < k, so exp gives
        # exact zeros (the DVE fast-exp int16 convert saturates to -0.0).
        nc.gpsimd.memset(trim_sb[:], -1.0e5)
        nc.gpsimd.affine_select(
            out=trim_sb[:],
            in_=trim_sb[:],
            pattern=[[1, 128]],
            compare_op=mybir.AluOpType.is_ge,
            fill=0.0,
            base=-1,
            channel_multiplier=-1,
        )
        nc.gpsimd.memset(ident_sb[:], 1.0)
        nc.gpsimd.affine_select(
            out=ident_sb[:],
            in_=ident_sb[:],
            pattern=[[1, 128]],
            compare_op=mybir.AluOpType.is_equal,
            fill=0.0,
            base=0,
            channel_multiplier=-1,
        )

        # --- qkv projections, g-block pipelined ---
        with tc.tile_pool(name="pqk", bufs=6, space="PSUM") as pqk:
            # warmup: dummy matmuls on a memset tile keep the PE busy during
            # the input DMA window so the HAM clock gate is warm (2.4 GHz)
            # when real work starts
            warm_sb = const.tile([128, 512], bf16, name="warm_sb")
            nc.vector.memset(warm_sb[:], 0.0)
            wps = pqk.tile([128, 512], f32, tag="mix", name="warmps")
            for w in range(25):
                nc.tensor.matmul(
                    wps[:],
                    lhsT=warm_sb[:, 0:128],
                    rhs=warm_sb[:],
                    start=True,
                    stop=True,
                    skip_group_check=True,
                )

            def qk_g(g):
                for m in range(2):
                    for wsb, dst, bsb in (
                        (wq_sb, qT_sb, bq_sb),
                        (wk_sb, kT_sb, bk_sb),
                    ):
                        ps = pqk.tile(
                            [128, 512], f32, tag="mix", name=f"pqk{g}_{m}_{id(wsb)}"
                        )
                        for i in range(NE):
                            nc.tensor.matmul(
                                ps[:],
                                lhsT=wsb[:, i, 128 * m : 128 * m + 128],
                                rhs=qt_sb[:, g, i, :],
                                start=(i == 0),
                                stop=(i == NE - 1),
                            )
                        nc.scalar.activation(
                            dst[:, m, 512 * g : 512 * g + 512],
                            ps[:],
                            Ident,
                            bias=bsb[:, m : m + 1],
                            scale=1.0,
                        )

            def v_si(si):
                ps = pqk.tile([128, 512], f32, tag="mix", name=f"pv{si}")
                for i in range(NE):
                    nc.tensor.matmul(
                        ps[:, 0:C],
                        lhsT=qt_sb[
                            :, si // 4, i, 128 * (si % 4) : 128 * (si % 4) + 128
                        ],
                        rhs=wv_sb[:, i, :],
                        start=(i == 0),
                        stop=False,
                    )
                nc.tensor.matmul(
                    ps[:, 0:C],
                    lhsT=ones_sb[:, 0:128],
                    rhs=bv_sb[:],
                    start=False,
                    stop=True,
                )
                nc.vector.tensor_copy(
                    v_sb[si][:, :, 0:D],
                    ps[:, 0:C].rearrange("p (h d) -> p h d", h=HC),
                )

            for g in range(4):
                qk_g(g)
                for si in range(4 * g, 4 * g + 4):
                    v_si(si)

        # --- attention: head pairs share one [128,1024] score tile
        # (head A cols 0:512, head B cols 512:1024) so a single exp call
        # covers both heads; emission is software-pipelined (scores of
        # unit i+1 issue before av of unit i) to keep the PE dense ---
        unit_idx = [0]
        with tc.tile_pool(name="psc", bufs=2, space="PSUM") as psc, tc.tile_pool(
            name="pav", bufs=4, space="PSUM"
        ) as pav, tc.tile_pool(name="ppool", bufs=6) as ppool, tc.tile_pool(
            name="rl", bufs=8
        ) as rl:

            # post-processing is split: phase 1 (a single ScalarE copy, done
            # at the accumulation stop) frees the PSUM bank fast; phase 2
            # (reciprocal/broadcast/normalize from scratch) is deferred and
            # drip-fed one per unit so it never clogs the DVE FIFO ahead of
            # the exp calls the PE is waiting on
            post_q = []

            def post_g2(pr, hb, av, gq):
                h = 2 * pr + hb
                avs = rl.tile([D + 1, 512], f32, tag="avs", name=f"avs{h}_{gq}")
                nc.scalar.copy(avs[:], av[0 : D + 1, :])
                post_q.append((pr, hb, gq, avs))

            def flush_post(keep=0):
                while len(post_q) > keep:
                    pr, hb, gq, avs = post_q.pop(0)
                    h = 2 * pr + hb
                    ltall = rl.tile(
                        [128, 4], f32, tag="lt", name=f"lt{h}_{gq}"
                    )
                    l_row = avs[D : D + 1, :]
                    nc.sync.dma_start(
                        ltall[:],
                        bass.AP(
                            tensor=l_row.tensor,
                            offset=l_row.offset,
                            ap=[list(l_row.ap[0]), [4, 128], [1, 4]],
                        ),
                    )
                    nc.vector.reciprocal(ltall[:], ltall[:])
                    ridx = 4 * h + gq
                    nc.sync.dma_start(
                        rrow[ridx, :].rearrange("(p c) -> p c", p=128), ltall[:]
                    )
                    rb = rl.tile([64, 512], f32, tag="rb", name=f"rb{h}_{gq}")
                    rr = rrow[ridx, :]
                    nc.sync.dma_start(
                        rb[:],
                        bass.AP(
                            tensor=rr.tensor,
                            offset=rr.offset,
                            ap=[[0, 64], [1, 512]],
                        ),
                    )
                    b0 = 64 * hb
                    nc.vector.tensor_mul(
                        out_sb[b0 : b0 + 64, pr, 512 * gq : 512 * gq + 512],
                        avs[0:D, :],
                        rb[:],
                    )

            # av matmuls run two units behind their exp so their semaphores
            # are always pre-satisfied and the PE stays fully pipelined
            pending = []

            def flush_av(keep=0):
                while len(pending) > keep:
                    pr, Hh, kc, g2, pt, pavt = pending.pop(0)
                    stop_kc = 8 * Hh + (3 if g2 == 0 else 7)
                    for hb in range(2):
                        nc.tensor.matmul(
                            pavt[hb][g2][:],
                            lhsT=v_sb[kc][:, 2 * pr + hb, :],
                            rhs=pt[:, 512 * hb : 512 * hb + 512],
                            start=(kc == 0),
                            stop=(kc == stop_kc),
                        )
                        if kc == stop_kc:
                            post_g2(pr, hb, pavt[hb][g2], 2 * Hh + g2)

            for pr in range(2):
                for Hh in range(2):
                    q0 = 1024 * Hh
                    klast = 8 * Hh + 8
                    # av accumulators: [head(2)][g2(2)]
                    avt = [
                        [
                            pav.tile(
                                [128, 512],
                                f32,
                                tag="av",
                                name=f"av{pr}_{Hh}_{hb}_{g2}",
                            )
                            for g2 in range(2)
                        ]
                        for hb in range(2)
                    ]
                    for kc in range(klast):
                        md = kc - 8 * Hh
                        for g2 in range(2):
                            if md >= 4 and g2 == 0:
                                continue
                            qc0 = q0 + 512 * g2
                            # diagonal block bookkeeping (local cols in unit)
                            diag = 0 <= md <= 7 and g2 == md // 4
                            ds = 128 * md - 512 * g2 if diag else 0
                            ps = psc.tile(
                                [128, 1024],
                                f32,
                                tag="sc",
                                name=f"sc{pr}_{Hh}_{g2}_{kc}",
                            )
                            # concurrent row-tiled score matmuls (adjacent):
                            # head A -> cols 0:512, head B -> cols 512:1024
                            for hb in range(2):
                                b0 = 64 * hb
                                nc.tensor.matmul(
                                    ps[:, 512 * hb : 512 * hb + 512],
                                    lhsT=kT_sb[
                                        b0 : b0 + 64,
                                        pr,
                                        128 * kc : 128 * kc + 128,
                                    ],
                                    rhs=qT_sb[b0 : b0 + 64, pr, qc0 : qc0 + 512],
                                    start=True,
                                    stop=not diag,
                                )
                            if diag:
                                # matmul-based causal mask on both diag blocks
                                for hb in range(2):
                                    nc.tensor.matmul(
                                        ps[
                                            :,
                                            512 * hb + ds : 512 * hb + ds + 128,
                                        ],
                                        lhsT=trim_sb[:],
                                        rhs=ident_sb[:],
                                        start=False,
                                        stop=True,
                                    )
                            # av lags by 2 units; flush the oldest
                            flush_av(keep=2)
                            pt = ppool.tile(
                                [128, 1024],
                                bf16,
                                tag="p",
                                name=f"pt{pr}_{Hh}_{g2}_{kc}",
                            )
                            if diag and ds > 0:
                                nc.gpsimd.memset(pt[:, 0:ds], 0.0)
                            # one exp covers both heads; alternate engines
                            u = unit_idx[0]
                            if u % 2 == 0:
                                nc.scalar.activation(
                                    pt[:, ds:1024], ps[:, ds:1024], Exp
                                )
                            else:
                                nc.vector.tensor_scalar(
                                    pt[:, ds:1024].bitcast(i16),
                                    ps[:, ds:1024],
                                    A16,
                                    B16C,
                                    Mult,
                                    Add,
                                )
                            if diag and ds > 0:
                                # B-half masked prefix sits inside the exp
                                # range; zero it after the exp has written it
                                nc.gpsimd.memset(pt[:, 512 : 512 + ds], 0.0)
                            flush_post(keep=5)
                            unit_idx[0] += 1
                            pending.append((pr, Hh, kc, g2, pt, avt))
            flush_av(keep=0)
            flush_post(keep=0)

        # --- output projection (partial: this core's 256 contraction rows) ---
        with tc.tile_pool(name="py", bufs=4, space="PSUM") as py, tc.tile_pool(
            name="ysb", bufs=4
        ) as ysb:
            for t in range(16):
                yt = ysb.tile([128, E], bf16, tag="yt", name=f"yt{t}")
                for e in range(2):
                    ps = py.tile([128, 512], f32, tag="pj", name=f"py{t}_{e}")
                    for m in range(2):
                        nc.tensor.matmul(
                            ps[:],
                            lhsT=out_sb[:, m, 128 * t : 128 * t + 128],
                            rhs=wo_sb[:, m, 512 * e : 512 * e + 512],
                            start=(m == 0),
                            stop=(m == 1),
                        )
                    if e == 0:
                        nc.vector.tensor_copy(yt[:, 0:512], ps[:])
                    else:
                        nc.scalar.copy(yt[:, 512:1024], ps[:])
                [nc.sync, nc.scalar, nc.gpsimd][t % 3].dma_start(y[t, :, :], yt[:])


_NC = None


def build_nc():
    global _NC
    if _NC is not None:
        return _NC
    nc = bacc.Bacc("TRN2", target_bir_lowering=False, debug=False, num_devices=8)
    qt = nc.dram_tensor("qt", [4 * 128 * NE * 512], bf16, kind="ExternalInput").ap()
    wq = nc.dram_tensor("wq", [128 * NE * C], bf16, kind="ExternalInput").ap()
    wk = nc.dram_tensor("wk", [128 * NE * C], bf16, kind="ExternalInput").ap()
    wv = nc.dram_tensor("wv", [128 * NE * C], bf16, kind="ExternalInput").ap()
    wo = nc.dram_tensor("wo", [128 * 2 * E], bf16, kind="ExternalInput").ap()
    bq = nc.dram_tensor("bq", [128, 2], f32, kind="ExternalInput").ap()
    bk = nc.dram_tensor("bk", [128, 2], f32, kind="ExternalInput").ap()
    bv = nc.dram_tensor("bv", [1, C], bf16, kind="ExternalInput").ap()
    y = nc.dram_tensor("y", [16, 128, E], bf16, kind="ExternalOutput").ap()
    with tile.TileContext(nc) as tc:
        _build_kernel(tc, qt, wq, wk, wv, wo, bq, bk, bv, y)
    nc.compile()
    _NC = nc
    return nc


def make_in_maps(Q, Wqkv, bqkv, Wout):
    """Per-core input dicts (8 cores: batch-major, then head-group)."""
    in_maps = []
    for c in range(8):
        b, hq = c // 4, c % 4
        cs = C * hq
        # qt: S-major g-blocks [4, 128, NE, 512]
        qtt = np.ascontiguousarray(Q[b].T).reshape(NE, 128, S)
        qt_np = (
            np.stack(
                [
                    np.ascontiguousarray(
                        qtt[:, :, 512 * g : 512 * g + 512].transpose(1, 0, 2)
                    )
                    for g in range(4)
                ]
            )
            .astype(bfnp)
            .reshape(-1)
        )

        def packw(w):
            # [E, C] -> sbuf layout [128 p, NE, C] flattened
            return (
                np.ascontiguousarray(w.reshape(NE, 128, C).transpose(1, 0, 2))
                .astype(bfnp)
                .reshape(-1)
            )

        wq_np = packw(Wqkv[:, cs : cs + C] * 0.125)
        wk_np = packw(Wqkv[:, E + cs : E + cs + C])
        wv_np = packw(Wqkv[:, 2 * E + cs : 2 * E + cs + C])
        bq_np = np.ascontiguousarray(
            (bqkv[cs : cs + C].astype(np.float32) * 0.125).reshape(2, 128).T
        )
        bk_np = np.ascontiguousarray(
            bqkv[E + cs : E + cs + C].astype(np.float32).reshape(2, 128).T
        )
        bv_np = bqkv[2 * E + cs : 2 * E + cs + C].reshape(1, C).astype(bfnp)
        wo_np = (
            np.ascontiguousarray(
                Wout[cs : cs + C, :].reshape(2, 128, E).transpose(1, 0, 2)
            )
            .astype(bfnp)
            .reshape(-1)
        )
        in_maps.append(
            {
                "qt": qt_np,
                "wo": wo_np,
                "wq": wq_np,
                "wk": wk_np,
                "wv": wv_np,
                "bq": bq_np,
                "bk": bk_np,
                "bv": bv_np,
            }
        )
    return in_maps


def kernel(Q, Wqkv, bqkv, Wout, bout, _trace=False, _trace_kwargs=None):
    Q = np.asarray(Q, dtype=np.float32)
    Wqkv = np.asarray(Wqkv, dtype=np.float32)
    bqkv = np.asarray(bqkv, dtype=np.float32)
    Wout = np.asarray(Wout, dtype=np.float32)
    bout = np.asarray(bout, dtype=np.float32)

    nc = build_nc()
    in_maps = make_in_maps(Q, Wqkv, bqkv, Wout)

    kwargs = {}
    if _trace:
        kwargs = dict(trace=True, trace_cores=list(range(8)))
        if _trace_kwargs:
            kwargs.update(_trace_kwargs)
    res = run_bass_kernel_spmd(nc, in_maps, core_ids=list(range(8)), **kwargs)

    out = np.zeros((2, S, E), dtype=np.float32)
    for c in range(8):
        yc = np.asarray(res.results[c]["y"]).astype(np.float32).reshape(S, E)
        out[c // 4] += yc
    out += bout.astype(np.float32)[None, None, :]
    if _trace:
        kernel._last_results = res
    return out


# revision 46
# speedup vs baseline: 1.0556x; 1.0556x over previous
"""Multi-head self-attention (B=2, S=2048, E=1024, H=16, causal) on 8 trn2 cores.

Sharding: core c handles batch b = c // 4 and heads [4*(c%4), 4*(c%4)+4).
Each core computes its 4 heads' attention and a partial output projection
(row-sharded Wout); the host sums the 4 partials per batch and adds bout.

Key optimizations over the naive version:
- qt loaded S-major in four 1MB blocks so the first projection matmuls start
  ~6us in instead of waiting for the full 4MB.
- Attention processes head PAIRS: the two heads of a pair live in SBUF
  partitions 0-63 / 64-127, so their K=64 score matmuls land on different
  PE row-groups (tile_position (0,0) vs (64,0)) and stream concurrently.
- exp is split across ScalarE (exact activation) and VectorE (Schraudolph
  fast-exp: bits = round(x*128/ln2 + B) written as int16 = bf16 bits).
- V stationary padded to 128 columns (ones in 64..127) so LDWEIGHTS runs
  with FWL; column 64 doubles as the softmax-denominator ones column.
- Output projection partials stored/DMAd as bf16 (halves output traffic).
"""

import os
from contextlib import ExitStack

import ml_dtypes
import numpy as np

import concourse.bass as bass
import concourse.mybir as mybir
import concourse.tile as tile
from concourse import bacc
from concourse.bass_utils import run_bass_kernel_spmd

f32 = mybir.dt.float32
bf16 = mybir.dt.bfloat16
i16 = mybir.dt.int16
bfnp = ml_dtypes.bfloat16

S = 2048
E = 1024
HC = 4  # heads per core
D = 64
C = HC * D  # 256 per-core head dims
NE = E // 128  # 8 contraction chunks

Exp = mybir.ActivationFunctionType.Exp
Ident = mybir.ActivationFunctionType.Identity
Mult = mybir.AluOpType.mult
Add = mybir.AluOpType.add

# Schraudolph fast-exp constants in bf16-bit domain
A16 = 128.0 / np.log(2.0)
B16C = 127.0 * 128.0 - 7.0


def _build_kernel(tc, qt, wq, wk, wv, wo, bq, bk, y):
    nc = tc.nc
    rrow = nc.dram_tensor("rrow", [16, 512], f32).ap()
    with ExitStack() as ctx:
        const = ctx.enter_context(tc.tile_pool(name="const", bufs=1))
        qt_sb = const.tile([128, 4, NE, 512], bf16)
        wq_sb = const.tile([128, NE, C], bf16)
        wk_sb = const.tile([128, NE, C], bf16)
        wv_sb = const.tile([128, NE, C], bf16)
        wo_sb = const.tile([128, 2, E], bf16)
        bq_sb = const.tile([128, 2], f32)
        bk_sb = const.tile([128, 2], f32)
        trim_sb = const.tile([128, 128], bf16)
        ident_sb = const.tile([128, 128], bf16)
        qT_sb = const.tile([128, 2, S], bf16)
        kT_sb = const.tile([128, 2, S], bf16)
        v_sb = [
            const.tile([128, HC, 128], bf16, tag=f"v{si}", name=f"v_sb{si}")
            for si in range(16)
        ]
        out_sb = const.tile([128, 2, S], bf16)

        # --- loads: weights first (parallel queues), qt S-major on sync ---
        nc.scalar.dma_start(wq_sb[:], wq.rearrange("(p a) -> p a", p=128))
        nc.gpsimd.dma_start(wk_sb[:], wk.rearrange("(p a) -> p a", p=128))
        qt_r = qt.rearrange("(g p i c) -> g p i c", g=4, p=128, i=NE)
        nc.sync.dma_start(qt_sb[:, 0], qt_r[0])
        nc.sync.dma_start(bq_sb[:], bq[:])
        nc.sync.dma_start(bk_sb[:], bk[:])
        for g in range(1, 4):
            nc.sync.dma_start(qt_sb[:, g], qt_r[g])
        nc.scalar.dma_start(wv_sb[:], wv.rearrange("(p a) -> p a", p=128))
        nc.gpsimd.dma_start(wo_sb[:], wo.rearrange("(p a) -> p a", p=128))
        for si in range(16):
            nc.gpsimd.memset(v_sb[si][:, :, D:128], 1.0)
        # constant tiles for matmul-based causal masking of diagonal blocks:
        # trim[j, k] = -1e5 iff j < k; ident = I. Then trim.T @ I accumulated
        # into a 128-col score block adds -1e5 wherever q < k, so exp gives
        # exact zeros (the DVE fast-exp int16 convert saturates to -0.0).
        nc.gpsimd.memset(trim_sb[:], -1.0e5)
        nc.gpsimd.affine_select(
            out=trim_sb[:],
            in_=trim_sb[:],
            pattern=[[1, 128]],
            compare_op=mybir.AluOpType.is_ge,
            fill=0.0,
            base=-1,
            channel_multiplier=-1,
        )
        nc.gpsimd.memset(ident_sb[:], 1.0)
        nc.gpsimd.affine_select(
            out=ident_sb[:],
            in_=ident_sb[:],
            pattern=[[1, 128]],
            compare_op=mybir.AluOpType.is_equal,
            fill=0.0,
            base=0,
            channel_multiplier=-1,
        )

        # --- qkv projections, g-block pipelined ---
        with tc.tile_pool(name="pqk", bufs=6, space="PSUM") as pqk:
            # warmup: dummy matmuls on a memset tile keep the PE busy during
            # the input DMA window so the HAM clock gate is warm (2.4 GHz)
            # when real work starts
            warm_sb = const.tile([128, 512], bf16, name="warm_sb")
            nc.vector.memset(warm_sb[:], 0.0)
            wps = pqk.tile([128, 512], f32, tag="mix", name="warmps")
            for w in range(25):
                nc.tensor.matmul(
                    wps[:],
                    lhsT=warm_sb[:, 0:128],
                    rhs=warm_sb[:],
                    start=True,
                    stop=True,
                    skip_group_check=True,
                )

            def qk_g(g):
                for m in range(2):
                    for wsb, dst, bsb in (
                        (wq_sb, qT_sb, bq_sb),
                        (wk_sb, kT_sb, bk_sb),
                    ):
                        ps = pqk.tile(
                            [128, 512], f32, tag="mix", name=f"pqk{g}_{m}_{id(wsb)}"
                        )
                        for i in range(NE):
                            nc.tensor.matmul(
                                ps[:],
                                lhsT=wsb[:, i, 128 * m : 128 * m + 128],
                                rhs=qt_sb[:, g, i, :],
                                start=(i == 0),
                                stop=(i == NE - 1),
                            )
                        nc.scalar.activation(
                            dst[:, m, 512 * g : 512 * g + 512],
                            ps[:],
                            Ident,
                            bias=bsb[:, m : m + 1],
                            scale=1.0,
                        )

            def v_si(si):
                # v bias is NOT applied here: attn rows sum to 1, so the
                # V-bias contribution to y is the constant row bv @ Wout,
                # which the host folds into bout
                ps = pqk.tile([128, 512], f32, tag="mix", name=f"pv{si}")
                for i in range(NE):
                    nc.tensor.matmul(
                        ps[:, 0:C],
                        lhsT=qt_sb[
                            :, si // 4, i, 128 * (si % 4) : 128 * (si % 4) + 128
                        ],
                        rhs=wv_sb[:, i, :],
                        start=(i == 0),
                        stop=(i == NE - 1),
                    )
                nc.vector.tensor_copy(
                    v_sb[si][:, :, 0:D],
                    ps[:, 0:C].rearrange("p (h d) -> p h d", h=HC),
                )

            for g in range(4):
                qk_g(g)
                for si in range(4 * g, 4 * g + 4):
                    v_si(si)

        # --- attention: head pairs share one [128,1024] score tile
        # (head A cols 0:512, head B cols 512:1024) so a single exp call
        # covers both heads; emission is software-pipelined (scores of
        # unit i+1 issue before av of unit i) to keep the PE dense ---
        unit_idx = [0]
        with tc.tile_pool(name="psc", bufs=2, space="PSUM") as psc, tc.tile_pool(
            name="pav", bufs=4, space="PSUM"
        ) as pav, tc.tile_pool(name="ppool", bufs=6) as ppool, tc.tile_pool(
            name="rl", bufs=8
        ) as rl:

            # post-processing is split: phase 1 (a single ScalarE copy, done
            # at the accumulation stop) frees the PSUM bank fast; phase 2
            # (reciprocal/broadcast/normalize from scratch) is deferred and
            # drip-fed one per unit so it never clogs the DVE FIFO ahead of
            # the exp calls the PE is waiting on
            post_q = []

            def post_g2(pr, hb, av, gq):
                h = 2 * pr + hb
                avs = rl.tile([D + 1, 512], f32, tag="avs", name=f"avs{h}_{gq}")
                nc.scalar.copy(avs[:], av[0 : D + 1, :])
                post_q.append((pr, hb, gq, avs))

            def flush_post(keep=0):
                while len(post_q) > keep:
                    pr, hb, gq, avs = post_q.pop(0)
                    h = 2 * pr + hb
                    ltall = rl.tile(
                        [128, 4], f32, tag="lt", name=f"lt{h}_{gq}"
                    )
                    l_row = avs[D : D + 1, :]
                    nc.sync.dma_start(
                        ltall[:],
                        bass.AP(
                            tensor=l_row.tensor,
                            offset=l_row.offset,
                            ap=[list(l_row.ap[0]), [4, 128], [1, 4]],
                        ),
                    )
                    nc.vector.reciprocal(ltall[:], ltall[:])
                    ridx = 4 * h + gq
                    nc.sync.dma_start(
                        rrow[ridx, :].rearrange("(p c) -> p c", p=128), ltall[:]
                    )
                    rb = rl.tile([64, 512], f32, tag="rb", name=f"rb{h}_{gq}")
                    rr = rrow[ridx, :]
                    nc.sync.dma_start(
                        rb[:],
                        bass.AP(
                            tensor=rr.tensor,
                            offset=rr.offset,
                            ap=[[0, 64], [1, 512]],
                        ),
                    )
                    b0 = 64 * hb
                    nc.vector.tensor_mul(
                        out_sb[b0 : b0 + 64, pr, 512 * gq : 512 * gq + 512],
                        avs[0:D, :],
                        rb[:],
                    )

            # av matmuls run two units behind their exp so their semaphores
            # are always pre-satisfied and the PE stays fully pipelined
            pending = []

            def flush_av(keep=0):
                while len(pending) > keep:
                    pr, Hh, kc, g2, pt, pavt = pending.pop(0)
                    stop_kc = 8 * Hh + (3 if g2 == 0 else 7)
                    for hb in range(2):
                        nc.tensor.matmul(
                            pavt[hb][g2][:],
                            lhsT=v_sb[kc][:, 2 * pr + hb, :],
                            rhs=pt[:, 512 * hb : 512 * hb + 512],
                            start=(kc == 0),
                            stop=(kc == stop_kc),
                        )
                        if kc == stop_kc:
                            post_g2(pr, hb, pavt[hb][g2], 2 * Hh + g2)

            for pr in range(2):
                for Hh in range(2):
                    q0 = 1024 * Hh
                    klast = 8 * Hh + 8
                    # av accumulators: [head(2)][g2(2)]
                    avt = [
                        [
                            pav.tile(
                                [128, 512],
                                f32,
                                tag="av",
                                name=f"av{pr}_{Hh}_{hb}_{g2}",
                            )
                            for g2 in range(2)
                        ]
                        for hb in range(2)
                    ]
                    for kc in range(klast):
                        md = kc - 8 * Hh
                        for g2 in range(2):
                            if md >= 4 and g2 == 0:
                                continue
                            qc0 = q0 + 512 * g2
                            # diagonal block bookkeeping (local cols in unit)
                            diag = 0 <= md <= 7 and g2 == md // 4
                            ds = 128 * md - 512 * g2 if diag else 0
                            ps = psc.tile(
                                [128, 1024],
                                f32,
                                tag="sc",
                                name=f"sc{pr}_{Hh}_{g2}_{kc}",
                            )
                            # concurrent row-tiled score matmuls (adjacent):
                            # head A -> cols 0:512, head B -> cols 512:1024
                            for hb in range(2):
                                b0 = 64 * hb
                                nc.tensor.matmul(
                                    ps[:, 512 * hb : 512 * hb + 512],
                                    lhsT=kT_sb[
                                        b0 : b0 + 64,
                                        pr,
                                        128 * kc : 128 * kc + 128,
                                    ],
                                    rhs=qT_sb[b0 : b0 + 64, pr, qc0 : qc0 + 512],
                                    start=True,
                                    stop=not diag,
                                )
                            if diag:
                                # matmul-based causal mask on both diag blocks
                                for hb in range(2):
                                    nc.tensor.matmul(
                                        ps[
                                            :,
                                            512 * hb + ds : 512 * hb + ds + 128,
                                        ],
                                        lhsT=trim_sb[:],
                                        rhs=ident_sb[:],
                                        start=False,
                                        stop=True,
                                    )
                            # av lags by 2 units; flush the oldest
                            flush_av(keep=2)
                            pt = ppool.tile(
                                [128, 1024],
                                bf16,
                                tag="p",
                                name=f"pt{pr}_{Hh}_{g2}_{kc}",
                            )
                            if diag and ds > 0:
                                nc.gpsimd.memset(pt[:, 0:ds], 0.0)
                            # one exp covers both heads; alternate engines
                            u = unit_idx[0]
                            if u % 2 == 0:
                                nc.scalar.activation(
                                    pt[:, ds:1024], ps[:, ds:1024], Exp
                                )
                            else:
                                nc.vector.tensor_scalar(
                                    pt[:, ds:1024].bitcast(i16),
                                    ps[:, ds:1024],
                                    A16,
                                    B16C,
                                    Mult,
                                    Add,
                                )
                            if diag and ds > 0:
                                # B-half masked prefix sits inside the exp
                                # range; zero it after the exp has written it
                                nc.gpsimd.memset(pt[:, 512 : 512 + ds], 0.0)
                            flush_post(keep=3)
                            unit_idx[0] += 1
                            pending.append((pr, Hh, kc, g2, pt, avt))
            flush_av(keep=0)
            flush_post(keep=0)

        # --- output projection (partial: this core's 256 contraction rows) ---
        with tc.tile_pool(name="py", bufs=4, space="PSUM") as py, tc.tile_pool(
            name="ysb", bufs=4
        ) as ysb:
            for t in range(16):
                yt = ysb.tile([128, E], bf16, tag="yt", name=f"yt{t}")
                for e in range(2):
                    ps = py.tile([128, 512], f32, tag="pj", name=f"py{t}_{e}")
                    for m in range(2):
                        nc.tensor.matmul(
                            ps[:],
                            lhsT=out_sb[:, m, 128 * t : 128 * t + 128],
                            rhs=wo_sb[:, m, 512 * e : 512 * e + 512],
                            start=(m == 0),
                            stop=(m == 1),
                        )
                    if e == 0:
                        nc.vector.tensor_copy(yt[:, 0:512], ps[:])
                    else:
                        nc.scalar.copy(yt[:, 512:1024], ps[:])
                engs = [nc.sync, nc.scalar, nc.gpsimd]
                engs[(2 * t) % 3].dma_start(y[t, :, 0:512], yt[:, 0:512])
                engs[(2 * t + 1) % 3].dma_start(y[t, :, 512:1024], yt[:, 512:1024])


_NC = None


def build_nc():
    global _NC
    if _NC is not None:
        return _NC
    nc = bacc.Bacc("TRN2", target_bir_lowering=False, debug=False, num_devices=8)
    qt = nc.dram_tensor("qt", [4 * 128 * NE * 512], bf16, kind="ExternalInput").ap()
    wq = nc.dram_tensor("wq", [128 * NE * C], bf16, kind="ExternalInput").ap()
    wk = nc.dram_tensor("wk", [128 * NE * C], bf16, kind="ExternalInput").ap()
    wv = nc.dram_tensor("wv", [128 * NE * C], bf16, kind="ExternalInput").ap()
    wo = nc.dram_tensor("wo", [128 * 2 * E], bf16, kind="ExternalInput").ap()
    bq = nc.dram_tensor("bq", [128, 2], f32, kind="ExternalInput").ap()
    bk = nc.dram_tensor("bk", [128, 2], f32, kind="ExternalInput").ap()
    y = nc.dram_tensor("y", [16, 128, E], bf16, kind="ExternalOutput").ap()
    with tile.TileContext(nc) as tc:
        _build_kernel(tc, qt, wq, wk, wv, wo, bq, bk, y)
    nc.compile()
    _NC = nc
    return nc


def make_in_maps(Q, Wqkv, bqkv, Wout):
    """Per-core input dicts (8 cores: batch-major, then head-group)."""
    in_maps = []
    for c in range(8):
        b, hq = c // 4, c % 4
        cs = C * hq
        # qt: S-major g-blocks [4, 128, NE, 512]
        qtt = np.ascontiguousarray(Q[b].T).reshape(NE, 128, S)
        qt_np = (
            np.stack(
                [
                    np.ascontiguousarray(
                        qtt[:, :, 512 * g : 512 * g + 512].transpose(1, 0, 2)
                    )
                    for g in range(4)
                ]
            )
            .astype(bfnp)
            .reshape(-1)
        )

        def packw(w):
            # [E, C] -> sbuf layout [128 p, NE, C] flattened
            return (
                np.ascontiguousarray(w.reshape(NE, 128, C).transpose(1, 0, 2))
                .astype(bfnp)
                .reshape(-1)
            )

        wq_np = packw(Wqkv[:, cs : cs + C] * 0.125)
        wk_np = packw(Wqkv[:, E + cs : E + cs + C])
        wv_np = packw(Wqkv[:, 2 * E + cs : 2 * E + cs + C])
        bq_np = np.ascontiguousarray(
            (bqkv[cs : cs + C].astype(np.float32) * 0.125).reshape(2, 128).T
        )
        bk_np = np.ascontiguousarray(
            bqkv[E + cs : E + cs + C].astype(np.float32).reshape(2, 128).T
        )
        wo_np = (
            np.ascontiguousarray(
                Wout[cs : cs + C, :].reshape(2, 128, E).transpose(1, 0, 2)
            )
            .astype(bfnp)
            .reshape(-1)
        )
        in_maps.append(
            {
                "qt": qt_np,
                "wo": wo_np,
                "wq": wq_np,
                "wk": wk_np,
                "wv": wv_np,
                "bq": bq_np,
                "bk": bk_np,
            }
        )
    return in_maps


def kernel(Q, Wqkv, bqkv, Wout, bout, _trace=False, _trace_kwargs=None):
    Q = np.asarray(Q, dtype=np.float32)
    Wqkv = np.asarray(Wqkv, dtype=np.float32)
    bqkv = np.asarray(bqkv, dtype=np.float32)
    Wout = np.asarray(Wout, dtype=np.float32)
    bout = np.asarray(bout, dtype=np.float32)

    nc = build_nc()
    in_maps = make_in_maps(Q, Wqkv, bqkv, Wout)

    kwargs = {}
    if _trace:
        kwargs = dict(trace=True, trace_cores=list(range(8)))
        if _trace_kwargs:
            kwargs.update(_trace_kwargs)
    res = run_bass_kernel_spmd(nc, in_maps, core_ids=list(range(8)), **kwargs)

    out = np.zeros((2, S, E), dtype=np.float32)
    for c in range(8):
        yc = np.asarray(res.results[c]["y"]).astype(np.float32).reshape(S, E)
        out[c // 4] += yc
    # V-bias folds into a constant output row: attn rows sum to 1, so the
    # +bv contribution to y is exactly bv @ Wout
    const_row = bqkv[2 * E : 3 * E].astype(np.float32) @ Wout.astype(np.float32)
    out += (const_row + bout.astype(np.float32))[None, None, :]
    if _trace:
        kernel._last_results = res
    return out


# revision 49
# speedup vs baseline: 1.1002x; 1.0423x over previous
"""Multi-head self-attention (B=2, S=2048, E=1024, H=16, causal) on 8 trn2 cores.

Sharding: core c handles batch b = c // 4 and heads [4*(c%4), 4*(c%4)+4).
Each core computes its 4 heads' attention and a partial output projection
(row-sharded Wout); the host sums the 4 partials per batch and adds bout.

Key optimizations over the naive version:
- qt loaded S-major in four 1MB blocks so the first projection matmuls start
  ~6us in instead of waiting for the full 4MB.
- Attention processes head PAIRS: the two heads of a pair live in SBUF
  partitions 0-63 / 64-127, so their K=64 score matmuls land on different
  PE row-groups (tile_position (0,0) vs (64,0)) and stream concurrently.
- exp is split across ScalarE (exact activation) and VectorE (Schraudolph
  fast-exp: bits = round(x*128/ln2 + B) written as int16 = bf16 bits).
- V stationary padded to 128 columns (ones in 64..127) so LDWEIGHTS runs
  with FWL; column 64 doubles as the softmax-denominator ones column.
- Output projection partials stored/DMAd as bf16 (halves output traffic).
"""

import os
from contextlib import ExitStack

import ml_dtypes
import numpy as np

import concourse.bass as bass
import concourse.mybir as mybir
import concourse.tile as tile
from concourse import bacc
from concourse.bass_utils import run_bass_kernel_spmd

f32 = mybir.dt.float32
bf16 = mybir.dt.bfloat16
i16 = mybir.dt.int16
bfnp = ml_dtypes.bfloat16

S = 2048
E = 1024
HC = 4  # heads per core
D = 64
C = HC * D  # 256 per-core head dims
NE = E // 128  # 8 contraction chunks

Exp = mybir.ActivationFunctionType.Exp
Ident = mybir.ActivationFunctionType.Identity
Mult = mybir.AluOpType.mult
Add = mybir.AluOpType.add

# Schraudolph fast-exp constants in bf16-bit domain
A16 = 128.0 / np.log(2.0)
B16C = 127.0 * 128.0 - 7.0


def _build_kernel(tc, qt, wq, wk, wv, wo, bq, bk, y):
    nc = tc.nc
    rrow = nc.dram_tensor("rrow", [16, 512], f32).ap()
    with ExitStack() as ctx:
        const = ctx.enter_context(tc.tile_pool(name="const", bufs=1))
        qt_sb = const.tile([128, 4, NE, 512], bf16)
        wq_sb = const.tile([128, NE, C], bf16)
        wk_sb = const.tile([128, NE, C], bf16)
        wv_sb = const.tile([128, NE, C], bf16)
        wo_sb = const.tile([128, 2, E], bf16)
        bq_sb = const.tile([128, 2], f32)
        bk_sb = const.tile([128, 2], f32)
        trim_sb = const.tile([128, 128], bf16)
        ident_sb = const.tile([128, 128], bf16)
        qT_sb = const.tile([128, 2, S], bf16)
        kT_sb = const.tile([128, 2, S], bf16)
        v_sb = [
            const.tile([128, HC, 128], bf16, tag=f"v{si}", name=f"v_sb{si}")
            for si in range(16)
        ]
        out_sb = const.tile([128, 2, S], bf16)

        # --- loads: weights first (parallel queues), qt S-major on sync ---
        nc.scalar.dma_start(wq_sb[:], wq.rearrange("(p a) -> p a", p=128))
        nc.gpsimd.dma_start(wk_sb[:], wk.rearrange("(p a) -> p a", p=128))
        qt_r = qt.rearrange("(g p i c) -> g p i c", g=4, p=128, i=NE)
        nc.sync.dma_start(qt_sb[:, 0], qt_r[0])
        nc.sync.dma_start(bq_sb[:], bq[:])
        nc.sync.dma_start(bk_sb[:], bk[:])
        for g in range(1, 4):
            nc.sync.dma_start(qt_sb[:, g], qt_r[g])
        nc.scalar.dma_start(wv_sb[:], wv.rearrange("(p a) -> p a", p=128))
        nc.gpsimd.dma_start(wo_sb[:], wo.rearrange("(p a) -> p a", p=128))
        for si in range(16):
            nc.gpsimd.memset(v_sb[si][:, :, D:128], 1.0)
        # constant tiles for matmul-based causal masking of diagonal blocks:
        # trim[j, k] = -1e5 iff j < k; ident = I. Then trim.T @ I accumulated
        # into a 128-col score block adds -1e5 wherever q < k, so exp gives
        # exact zeros (the DVE fast-exp int16 convert saturates to -0.0).
        nc.gpsimd.memset(trim_sb[:], -1.0e5)
        nc.gpsimd.affine_select(
            out=trim_sb[:],
            in_=trim_sb[:],
            pattern=[[1, 128]],
            compare_op=mybir.AluOpType.is_ge,
            fill=0.0,
            base=-1,
            channel_multiplier=-1,
        )
        nc.gpsimd.memset(ident_sb[:], 1.0)
        nc.gpsimd.affine_select(
            out=ident_sb[:],
            in_=ident_sb[:],
            pattern=[[1, 128]],
            compare_op=mybir.AluOpType.is_equal,
            fill=0.0,
            base=0,
            channel_multiplier=-1,
        )

        # --- qkv projections, g-block pipelined ---
        with tc.tile_pool(name="pqk", bufs=6, space="PSUM") as pqk:
            # warmup: dummy matmuls on a memset tile keep the PE busy during
            # the input DMA window so the HAM clock gate is warm (2.4 GHz)
            # when real work starts
            warm_sb = const.tile([128, 512], bf16, name="warm_sb")
            nc.vector.memset(warm_sb[:], 0.0)
            wps = pqk.tile([128, 512], f32, tag="mix", name="warmps")
            for w in range(25):
                nc.tensor.matmul(
                    wps[:],
                    lhsT=warm_sb[:, 0:128],
                    rhs=warm_sb[:],
                    start=True,
                    stop=True,
                    skip_group_check=True,
                )

            def qk_g(g):
                for m in range(2):
                    for wsb, dst, bsb in (
                        (wq_sb, qT_sb, bq_sb),
                        (wk_sb, kT_sb, bk_sb),
                    ):
                        ps = pqk.tile(
                            [128, 512], f32, tag="mix", name=f"pqk{g}_{m}_{id(wsb)}"
                        )
                        for i in range(NE):
                            nc.tensor.matmul(
                                ps[:],
                                lhsT=wsb[:, i, 128 * m : 128 * m + 128],
                                rhs=qt_sb[:, g, i, :],
                                start=(i == 0),
                                stop=(i == NE - 1),
                            )
                        nc.scalar.activation(
                            dst[:, m, 512 * g : 512 * g + 512],
                            ps[:],
                            Ident,
                            bias=bsb[:, m : m + 1],
                            scale=1.0,
                        )

            def v_si(si):
                # v bias is NOT applied here: attn rows sum to 1, so the
                # V-bias contribution to y is the constant row bv @ Wout,
                # which the host folds into bout
                ps = pqk.tile([128, 512], f32, tag="mix", name=f"pv{si}")
                for i in range(NE):
                    nc.tensor.matmul(
                        ps[:, 0:C],
                        lhsT=qt_sb[
                            :, si // 4, i, 128 * (si % 4) : 128 * (si % 4) + 128
                        ],
                        rhs=wv_sb[:, i, :],
                        start=(i == 0),
                        stop=(i == NE - 1),
                    )
                nc.vector.tensor_copy(
                    v_sb[si][:, :, 0:D],
                    ps[:, 0:C].rearrange("p (h d) -> p h d", h=HC),
                )

            for g in range(4):
                qk_g(g)
                for si in range(4 * g, 4 * g + 4):
                    v_si(si)

        # --- attention: head pairs share one [128,1024] score tile
        # (head A cols 0:512, head B cols 512:1024) so a single exp call
        # covers both heads; emission is software-pipelined (scores of
        # unit i+1 issue before av of unit i) to keep the PE dense ---
        unit_idx = [0]
        with tc.tile_pool(name="psc", bufs=3, space="PSUM") as psc, tc.tile_pool(
            name="pav", bufs=2, space="PSUM"
        ) as pav, tc.tile_pool(name="ppool", bufs=6) as ppool, tc.tile_pool(
            name="rl", bufs=8
        ) as rl:

            # post-processing is split: phase 1 (a single ScalarE copy, done
            # at the accumulation stop) frees the PSUM bank fast; phase 2
            # (reciprocal/broadcast/normalize from scratch) is deferred and
            # drip-fed one per unit so it never clogs the DVE FIFO ahead of
            # the exp calls the PE is waiting on
            post_q = []

            def post_g2(pr, hb, av, gq):
                h = 2 * pr + hb
                avs = rl.tile([D + 1, 512], f32, tag="avs", name=f"avs{h}_{gq}")
                nc.scalar.copy(avs[:], av[0 : D + 1, :])
                post_q.append((pr, hb, gq, avs))

            def flush_post(keep=0):
                while len(post_q) > keep:
                    pr, hb, gq, avs = post_q.pop(0)
                    h = 2 * pr + hb
                    ltall = rl.tile(
                        [128, 4], f32, tag="lt", name=f"lt{h}_{gq}"
                    )
                    l_row = avs[D : D + 1, :]
                    nc.sync.dma_start(
                        ltall[:],
                        bass.AP(
                            tensor=l_row.tensor,
                            offset=l_row.offset,
                            ap=[list(l_row.ap[0]), [4, 128], [1, 4]],
                        ),
                    )
                    nc.vector.reciprocal(ltall[:], ltall[:])
                    ridx = 4 * h + gq
                    nc.sync.dma_start(
                        rrow[ridx, :].rearrange("(p c) -> p c", p=128), ltall[:]
                    )
                    rb = rl.tile([64, 512], f32, tag="rb", name=f"rb{h}_{gq}")
                    rr = rrow[ridx, :]
                    nc.sync.dma_start(
                        rb[:],
                        bass.AP(
                            tensor=rr.tensor,
                            offset=rr.offset,
                            ap=[[0, 64], [1, 512]],
                        ),
                    )
                    b0 = 64 * hb
                    nc.vector.tensor_mul(
                        out_sb[b0 : b0 + 64, pr, 512 * gq : 512 * gq + 512],
                        avs[0:D, :],
                        rb[:],
                    )

            # av matmuls run two units behind their exp so their semaphores
            # are always pre-satisfied and the PE stays fully pipelined
            pending = []

            def flush_av(keep=0):
                while len(pending) > keep:
                    pr, Hh, kc, g2, pt, pavt = pending.pop(0)
                    stop_kc = 8 * Hh + (3 if g2 == 0 else 7)
                    for hb in range(2):
                        nc.tensor.matmul(
                            pavt[hb][:],
                            lhsT=v_sb[kc][:, 2 * pr + hb, :],
                            rhs=pt[:, 512 * hb : 512 * hb + 512],
                            start=(kc == 0),
                            stop=(kc == stop_kc),
                        )
                        if kc == stop_kc:
                            post_g2(pr, hb, pavt[hb], 2 * Hh + g2)

            for pr in range(2):
                for Hh in range(2):
                    q0 = 1024 * Hh
                    for g2 in range(2):
                        qc0 = q0 + 512 * g2
                        stop_kc = 8 * Hh + (3 if g2 == 0 else 7)
                        # av accumulators for this g2 pass: [head(2)]
                        avt = [
                            pav.tile(
                                [128, 512],
                                f32,
                                tag="av",
                                name=f"av{pr}_{Hh}_{g2}_{hb}",
                            )
                            for hb in range(2)
                        ]
                        for kc in range(stop_kc + 1):
                            md = kc - 8 * Hh
                            # diagonal block bookkeeping (local cols in unit)
                            diag = 0 <= md <= 7 and g2 == md // 4
                            ds = 128 * md - 512 * g2 if diag else 0
                            ps = psc.tile(
                                [128, 1024],
                                f32,
                                tag="sc",
                                name=f"sc{pr}_{Hh}_{g2}_{kc}",
                            )
                            # concurrent row-tiled score matmuls (adjacent):
                            # head A -> cols 0:512, head B -> cols 512:1024
                            for hb in range(2):
                                b0 = 64 * hb
                                nc.tensor.matmul(
                                    ps[:, 512 * hb : 512 * hb + 512],
                                    lhsT=kT_sb[
                                        b0 : b0 + 64,
                                        pr,
                                        128 * kc : 128 * kc + 128,
                                    ],
                                    rhs=qT_sb[b0 : b0 + 64, pr, qc0 : qc0 + 512],
                                    start=True,
                                    stop=not diag,
                                )
                            if diag:
                                # matmul-based causal mask on both diag blocks
                                for hb in range(2):
                                    nc.tensor.matmul(
                                        ps[
                                            :,
                                            512 * hb + ds : 512 * hb + ds + 128,
                                        ],
                                        lhsT=trim_sb[:],
                                        rhs=ident_sb[:],
                                        start=False,
                                        stop=True,
                                    )
                            # av lags by 2 units; flush the oldest
                            flush_av(keep=2)
                            pt = ppool.tile(
                                [128, 1024],
                                bf16,
                                tag="p",
                                name=f"pt{pr}_{Hh}_{g2}_{kc}",
                            )
                            if diag and ds > 0:
                                nc.gpsimd.memset(pt[:, 0:ds], 0.0)
                            # one exp covers both heads; alternate engines
                            u = unit_idx[0]
                            if u % 2 == 0:
                                nc.scalar.activation(
                                    pt[:, ds:1024], ps[:, ds:1024], Exp
                                )
                            else:
                                nc.vector.tensor_scalar(
                                    pt[:, ds:1024].bitcast(i16),
                                    ps[:, ds:1024],
                                    A16,
                                    B16C,
                                    Mult,
                                    Add,
                                )
                            if diag and ds > 0:
                                # B-half masked prefix sits inside the exp
                                # range; zero it after the exp has written it
                                nc.gpsimd.memset(pt[:, 512 : 512 + ds], 0.0)
                            flush_post(keep=3)
                            unit_idx[0] += 1
                            pending.append((pr, Hh, kc, g2, pt, avt))
            flush_av(keep=0)
            flush_post(keep=0)

        # --- output projection (partial: this core's 256 contraction rows) ---
        with tc.tile_pool(name="py", bufs=4, space="PSUM") as py, tc.tile_pool(
            name="ysb", bufs=4
        ) as ysb:
            for t in range(16):
                yt = ysb.tile([128, E], bf16, tag="yt", name=f"yt{t}")
                for e in range(2):
                    ps = py.tile([128, 512], f32, tag="pj", name=f"py{t}_{e}")
                    for m in range(2):
                        nc.tensor.matmul(
                            ps[:],
                            lhsT=out_sb[:, m, 128 * t : 128 * t + 128],
                            rhs=wo_sb[:, m, 512 * e : 512 * e + 512],
                            start=(m == 0),
                            stop=(m == 1),
                        )
                    if e == 0:
                        nc.vector.tensor_copy(yt[:, 0:512], ps[:])
                    else:
                        nc.scalar.copy(yt[:, 512:1024], ps[:])
                engs = [nc.sync, nc.scalar, nc.gpsimd]
                engs[(2 * t) % 3].dma_start(y[t, :, 0:512], yt[:, 0:512])
                engs[(2 * t + 1) % 3].dma_start(y[t, :, 512:1024], yt[:, 512:1024])


_NC = None


def build_nc():
    global _NC
    if _NC is not None:
        return _NC
    nc = bacc.Bacc("TRN2", target_bir_lowering=False, debug=False, num_devices=8)
    qt = nc.dram_tensor("qt", [4 * 128 * NE * 512], bf16, kind="ExternalInput").ap()
    wq = nc.dram_tensor("wq", [128 * NE * C], bf16, kind="ExternalInput").ap()
    wk = nc.dram_tensor("wk", [128 * NE * C], bf16, kind="ExternalInput").ap()
    wv = nc.dram_tensor("wv", [128 * NE * C], bf16, kind="ExternalInput").ap()
    wo = nc.dram_tensor("wo", [128 * 2 * E], bf16, kind="ExternalInput").ap()
    bq = nc.dram_tensor("bq", [128, 2], f32, kind="ExternalInput").ap()
    bk = nc.dram_tensor("bk", [128, 2], f32, kind="ExternalInput").ap()
    y = nc.dram_tensor("y", [16, 128, E], bf16, kind="ExternalOutput").ap()
    with tile.TileContext(nc) as tc:
        _build_kernel(tc, qt, wq, wk, wv, wo, bq, bk, y)
    nc.compile()
    _NC = nc
    return nc


def make_in_maps(Q, Wqkv, bqkv, Wout):
    """Per-core input dicts (8 cores: batch-major, then head-group)."""
    in_maps = []
    for c in range(8):
        b, hq = c // 4, c % 4
        cs = C * hq
        # qt: S-major g-blocks [4, 128, NE, 512]
        qtt = np.ascontiguousarray(Q[b].T).reshape(NE, 128, S)
        qt_np = (
            np.stack(
                [
                    np.ascontiguousarray(
                        qtt[:, :, 512 * g : 512 * g + 512].transpose(1, 0, 2)
                    )
                    for g in range(4)
                ]
            )
            .astype(bfnp)
            .reshape(-1)
        )

        def packw(w):
            # [E, C] -> sbuf layout [128 p, NE, C] flattened
            return (
                np.ascontiguousarray(w.reshape(NE, 128, C).transpose(1, 0, 2))
                .astype(bfnp)
                .reshape(-1)
            )

        wq_np = packw(Wqkv[:, cs : cs + C] * 0.125)
        wk_np = packw(Wqkv[:, E + cs : E + cs + C])
        wv_np = packw(Wqkv[:, 2 * E + cs : 2 * E + cs + C])
        bq_np = np.ascontiguousarray(
            (bqkv[cs : cs + C].astype(np.float32) * 0.125).reshape(2, 128).T
        )
        bk_np = np.ascontiguousarray(
            bqkv[E + cs : E + cs + C].astype(np.float32).reshape(2, 128).T
        )
        wo_np = (
            np.ascontiguousarray(
                Wout[cs : cs + C, :].reshape(2, 128, E).transpose(1, 0, 2)
            )
            .astype(bfnp)
            .reshape(-1)
        )
        in_maps.append(
            {
                "qt": qt_np,
                "wo": wo_np,
                "wq": wq_np,
                "wk": wk_np,
                "wv": wv_np,
                "bq": bq_np,
                "bk": bk_np,
            }
        )
    return in_maps


def kernel(Q, Wqkv, bqkv, Wout, bout, _trace=False, _trace_kwargs=None):
    Q = np.asarray(Q, dtype=np.float32)
    Wqkv = np.asarray(Wqkv, dtype=np.float32)
    bqkv = np.asarray(bqkv, dtype=np.float32)
    Wout = np.asarray(Wout, dtype=np.float32)
    bout = np.asarray(bout, dtype=np.float32)

    nc = build_nc()
    in_maps = make_in_maps(Q, Wqkv, bqkv, Wout)

    kwargs = {}
    if _trace:
        kwargs = dict(trace=True, trace_cores=list(range(8)))
        if _trace_kwargs:
            kwargs.update(_trace_kwargs)
    res = run_bass_kernel_spmd(nc, in_maps, core_ids=list(range(8)), **kwargs)

    out = np.zeros((2, S, E), dtype=np.float32)
    for c in range(8):
        yc = np.asarray(res.results[c]["y"]).astype(np.float32).reshape(S, E)
        out[c // 4] += yc
    # V-bias folds into a constant output row: attn rows sum to 1, so the
    # +bv contribution to y is exactly bv @ Wout
    const_row = bqkv[2 * E : 3 * E].astype(np.float32) @ Wout.astype(np.float32)
    out += (const_row + bout.astype(np.float32))[None, None, :]
    if _trace:
        kernel._last_results = res
    return out
